# revision 4
# baseline (speedup 1.0000x reference)
"""ALIGN module kernel for 8 TRN2 NeuronCores (vocab-parallel).

Reference computation (B=4, S=576, Dv=1024, Dl=4096, V=32000):
    x  = vision_feats @ W1_w.T + W1_b          # [T=2304, Dl]
    xn = layernorm(x)                          # over Dl, no affine
    P  = softmax(xn @ W2_w.T, axis=-1)         # [T, V]
    F  = P @ llm_token_embed                   # [T, Dl]

Sharding: vocab dim of W2_w / llm_token_embed split across the 8 cores
(4000 rows each, zero-padded to 4096). Stage A (W1 + LN) is token-parallel
with STRIDED ownership: core c owns tokens {768*si + 96*c + i} so each
768-token superblock is AllGathered separately (3 small AGs) and AG si+1
overlaps phase-B compute on superblock si — the AG latency that was fully
exposed in the single-AG version is hidden.

Stage A computes xT directly (x transposed: lhsT=W1T-tile, rhs=visionT) so
no PE transposes are needed; LayerNorm stats become column stats, computed
with ones-vector matmul reductions, and the per-column (per-token) mean /
rstd are broadcast across partitions with rank-1 matmuls, then applied on
DVE with stride-0 broadcast APs.

Softmax needs no max-subtraction (logits are ~N(0,1), |logit| < ~6): each
core computes exp(logits_loc); the 96 zero pad rows contribute exactly
exp(0)=1 each, masked out of the denominator by the ones_v mask. The
denominator rides e-group 0's ReduceScatter as an extra column; each core
only divides the 96-token slices it owns after the F ReduceScatter.

Host-side prep encodes all layout work: weights arrive pre-transposed,
pre-padded, pre-tiled for unit-stride DMA, and pre-cast to bf16.
"""

import os
import sys

for _p in ("/opt/trn_rl_repo", "/root/.axon_site/_ro/trn_rl_repo"):
    if os.path.isdir(_p) and _p not in sys.path:
        sys.path.insert(0, _p)

import numpy as np
import ml_dtypes

from concourse import bass, bacc, mybir, tile
from concourse.bass_utils import run_bass_kernel_spmd

BF16NP = ml_dtypes.bfloat16
F32 = mybir.dt.float32
BF16 = mybir.dt.bfloat16

N_CORES = 8
T = 2304          # total tokens (B*S)
DV = 1024
DL = 4096
V_PAD = 4096      # padded vocab rows per core (4000 real + 96 zero pads)
NVT = V_PAD // 128  # 32 vocab tiles per core
NJ = DL // 128      # 32 contraction tiles
NK = DV // 128      # 8 stage-A contraction tiles

# token superblocks; each is AllGathered separately and owned strided
SBS = [(0, 768), (768, 768), (1536, 768)]
TSB_MAX = 768
TCH = TSB_MAX // N_CORES   # 96 tokens per core per superblock (stage A)
T_LOC = TCH * len(SBS)     # 288 tokens per core total
C1S = [(0, 512), (512, 256)]  # matmul1 chunking of a superblock
EC = 512          # matmul2 embedding-chunk width (SBUF tile)
N_EC = DL // EC   # 8 e-chunks
EG = 2            # e-chunks per ReduceScatter group
EGW = EC * EG     # 1024 columns per RS

_NC_CACHE = None


def build():
    nc = bacc.Bacc("TRN2", target_bir_lowering=False, debug=False,
                   num_devices=N_CORES)
    rg = [list(range(N_CORES))]

    visionT = nc.dram_tensor("visionT", [DV, T_LOC], BF16, kind="ExternalInput")
    w1t = nc.dram_tensor("w1t", [DV, DL], BF16, kind="ExternalInput")
    w1b = nc.dram_tensor("w1b", [1, DL], BF16, kind="ExternalInput")
    # [vt][p][j][vi]: per-partition unit-stride 8KB runs
    w2t = nc.dram_tensor("w2t", [NVT, 128, NJ, 128], BF16, kind="ExternalInput")
    # [e][p][vt][n]: per-partition unit-stride 16KB runs
    emb = nc.dram_tensor("emb", [N_EC, 128, NVT, EC], BF16,
                         kind="ExternalInput")
    ones_v = nc.dram_tensor("ones_v", [128, NVT, 1], BF16, kind="ExternalInput")
    out = nc.dram_tensor("out", [T // N_CORES, DL], F32, kind="ExternalOutput")
    # last e-group of the last superblock skips its ReduceScatter: each core
    # ships its partial numerator (and denominator) and the host reduces.
    out2 = nc.dram_tensor("out2", [TSB_MAX, EGW], BF16, kind="ExternalOutput")
    out3 = nc.dram_tensor("out3", [TSB_MAX, 1], BF16, kind="ExternalOutput")

    from contextlib import ExitStack
    with tile.TileContext(nc) as tc, ExitStack() as ctx:
        consts = ctx.enter_context(tc.tile_pool(name="consts", bufs=1))
        dram = ctx.enter_context(tc.tile_pool(name="dram", bufs=1, space="DRAM"))
        dram_rs = ctx.enter_context(tc.tile_pool(name="dram_rs", bufs=4, space="DRAM"))
        # w2/eb pools live OUTSIDE stage A so their first loads prefetch
        # during stage A instead of waiting for the stage-A pools to free.
        w2_p = ctx.enter_context(tc.tile_pool(name="w2_p", bufs=3))
        eb_p = ctx.enter_context(tc.tile_pool(name="eb_p", bufs=2))
        if True:
            onesv_sb = consts.tile([128, NVT, 1], BF16)
            nc.sync.dma_start(onesv_sb, ones_v[:])

            # per-superblock AllGather buffers; byte layout of each block is
            # xnT row-major [DL, TCH] per core
            ag_ins = []
            ag_outs = []
            for si in range(len(SBS)):
                gi = dram.tile([DL * TCH], BF16, tag=f"ag_in_{si}",
                               name=f"ag_in_{si}")
                go = dram.tile([N_CORES * DL * TCH], BF16,
                               addr_space="Shared", tag=f"ag_out_{si}",
                               name=f"ag_out_{si}")
                ag_ins.append(gi)
                ag_outs.append(go)

            # ---------------- Stage A: xT = W1T.T-tiles @ visionT, LN cols
            with ExitStack() as actx:
                sa = actx.enter_context(tc.tile_pool(name="stageA", bufs=1))
                sa2 = actx.enter_context(tc.tile_pool(name="stageA2", bufs=2))
                psa = actx.enter_context(tc.tile_pool(name="psumA", bufs=1, space="PSUM"))

                vt_sb = sa.tile([128, NK, T_LOC], BF16)
                for k in range(NK):
                    nc.sync.dma_start(
                        vt_sb[:, k, :], visionT[128 * k:128 * (k + 1), :])
                w1t_sb = sa.tile([128, NK, DL], BF16)
                for k in range(NK):
                    nc.sync.dma_start(
                        w1t_sb[:, k, :], w1t[128 * k:128 * (k + 1), :])
                w1b_row = sa.tile([1, DL], BF16)
                nc.sync.dma_start(w1b_row, w1b[:])
                ones_t = sa.tile([1, TCH], BF16)
                nc.vector.memset(ones_t, 1.0)
                onescol = sa.tile([128, 1], BF16)
                nc.vector.memset(onescol, 1.0)
                onesrow_f = sa.tile([1, 128], F32)
                nc.vector.memset(onesrow_f, 1.0)
                eps_sc = sa.tile([1, 1], F32)
                nc.vector.memset(eps_sc, 1e-5)

                for si in range(len(SBS)):
                    c0 = TCH * si
                    # xT for this chunk: [dl-part, m, token]
                    xt = sa2.tile([128, NJ, TCH], BF16, tag="xt",
                                  name=f"xt_{si}")
                    for m in range(NJ):
                        xp = psa.tile([128, TCH], F32, tag="xp",
                                      name=f"xp_{si}_{m}", bufs=2)
                        for k in range(NK):
                            nc.tensor.matmul(
                                xp, lhsT=w1t_sb[:, k, 128 * m:128 * (m + 1)],
                                rhs=vt_sb[:, k, c0:c0 + TCH],
                                start=(k == 0), stop=False)
                        # rank-1 bias add: b[1,128m..].T @ ones[1,TCH]
                        nc.tensor.matmul(
                            xp, lhsT=w1b_row[:, 128 * m:128 * (m + 1)],
                            rhs=ones_t, start=False, stop=True)
                        nc.scalar.activation(
                            out=xt[:, m, :], in_=xp,
                            func=mybir.ActivationFunctionType.Identity)
                    sq = sa2.tile([128, NJ, TCH], BF16, tag="sq",
                                  name=f"sq_{si}")
                    nc.vector.tensor_mul(out=sq, in0=xt, in1=xt)
                    # column stats via ones-vector matmul reductions
                    s1p = psa.tile([1, TCH], F32, tag="s1", name=f"s1_{si}")
                    for m in range(NJ):
                        nc.tensor.matmul(s1p, lhsT=onescol, rhs=xt[:, m, :],
                                         start=(m == 0), stop=(m == NJ - 1))
                    s2p = psa.tile([1, TCH], F32, tag="s2", name=f"s2_{si}")
                    for m in range(NJ):
                        nc.tensor.matmul(s2p, lhsT=onescol, rhs=sq[:, m, :],
                                         start=(m == 0), stop=(m == NJ - 1))
                    mu_row = sa2.tile([1, TCH], F32, tag="mu",
                                      name=f"mu_{si}")
                    nc.vector.tensor_scalar(
                        out=mu_row, in0=s1p, scalar1=1.0 / DL, scalar2=None,
                        op0=mybir.AluOpType.mult)
                    msq_row = sa2.tile([1, TCH], F32, tag="msq",
                                       name=f"msq_{si}")
                    nc.vector.tensor_scalar(
                        out=msq_row, in0=s2p, scalar1=1.0 / DL, scalar2=None,
                        op0=mybir.AluOpType.mult)
                    var_row = sa2.tile([1, TCH], F32, tag="var",
                                       name=f"var_{si}")
                    # var = msq - mu^2
                    musq = sa2.tile([1, TCH], F32, tag="musq",
                                    name=f"musq_{si}")
                    nc.vector.tensor_mul(out=musq, in0=mu_row, in1=mu_row)
                    nc.vector.tensor_sub(out=var_row, in0=msq_row, in1=musq)
                    sd_row = sa2.tile([1, TCH], F32, tag="sd",
                                      name=f"sd_{si}")
                    nc.scalar.activation(
                        out=sd_row, in_=var_row,
                        func=mybir.ActivationFunctionType.Sqrt,
                        bias=eps_sc)
                    rstd_row = sa2.tile([1, TCH], F32, tag="rstd",
                                        name=f"rstd_{si}")
                    nc.vector.reciprocal(out=rstd_row, in_=sd_row)
                    # broadcast mu/rstd down the 128 partitions (rank-1 fp32)
                    mub_p = psa.tile([128, TCH], F32, tag="mub",
                                     name=f"mub_{si}")
                    nc.tensor.matmul(mub_p, lhsT=onesrow_f, rhs=mu_row)
                    rstdb_p = psa.tile([128, TCH], F32, tag="rstdb",
                                       name=f"rstdb_{si}")
                    nc.tensor.matmul(rstdb_p, lhsT=onesrow_f, rhs=rstd_row)
                    mub = sa2.tile([128, TCH], BF16, tag="mub_sb",
                                   name=f"mub_sb_{si}")
                    nc.vector.tensor_copy(out=mub, in_=mub_p)
                    rstdb = sa2.tile([128, TCH], BF16, tag="rstdb_sb",
                                     name=f"rstdb_sb_{si}")
                    nc.vector.tensor_copy(out=rstdb, in_=rstdb_p)

                    def bc(t):
                        # [128, TCH] -> [128, NJ, TCH] stride-0 broadcast
                        return bass.AP(
                            tensor=t.tensor, offset=t.offset,
                            ap=[list(t.ap[0]), [0, NJ], list(t.ap[-1])])

                    tmp = sa2.tile([128, NJ, TCH], BF16, tag="tmp",
                                   name=f"tmp_{si}")
                    nc.vector.tensor_sub(out=tmp, in0=xt, in1=bc(mub))
                    xn_ch = sa2.tile([128, NJ, TCH], BF16, tag="xn",
                                     name=f"xn_{si}")
                    nc.vector.tensor_mul(out=xn_ch, in0=tmp, in1=bc(rstdb))
                    nc.sync.dma_start(
                        bass.AP(tensor=ag_ins[si].tensor,
                                offset=ag_ins[si].offset,
                                ap=[[TCH, 128], [TCH * 128, NJ], [1, TCH]]),
                        xn_ch)
                    nc.gpsimd.collective_compute(
                        "AllGather", mybir.AluOpType.bypass,
                        replica_groups=rg,
                        ins=[ag_ins[si].opt()], outs=[ag_outs[si].opt()])

            # ---------------- Phase B
            xnt_p = ctx.enter_context(tc.tile_pool(name="xnt_p", bufs=1))
            pt_p = ctx.enter_context(tc.tile_pool(name="pt_p", bufs=1))
            fs_p = ctx.enter_context(tc.tile_pool(name="fs_p", bufs=2))
            fo_p = ctx.enter_context(tc.tile_pool(name="fo_p", bufs=1))
            small = ctx.enter_context(tc.tile_pool(name="small", bufs=2))
            l_ps = ctx.enter_context(tc.tile_pool(name="l_ps", bufs=3, space="PSUM"))
            s_ps = ctx.enter_context(tc.tile_pool(name="s_ps", bufs=2, space="PSUM"))
            f_ps = ctx.enter_context(tc.tile_pool(name="f_ps", bufs=3, space="PSUM"))
            if True:

                def make_xnt(si):
                    sb0, sbn = SBS[si]
                    # gather xnT for this superblock from the per-core
                    # blocks of ag_outs[si] (block c rows hold this SB's
                    # tokens 96c..96(c+1) as [d, t])
                    xnt = xnt_p.tile([128, NJ, TSB_MAX], BF16, tag="xnt",
                                     name=f"xnt_{si}")
                    for c in range(N_CORES):
                        for jb in range(0, NJ, 8):
                            off = (ag_outs[si].offset
                                   + (DL * c + 128 * jb) * TCH)
                            nc.sync.dma_start(
                                xnt[:, jb:jb + 8, TCH * c:TCH * (c + 1)],
                                bass.AP(tensor=ag_outs[si].tensor, offset=off,
                                        ap=[[TCH, 128], [TCH * 128, 8],
                                            [1, TCH]]))
                    return xnt

                xnt = make_xnt(0)
                for si, (sb0, sbn) in enumerate(SBS):
                    n_tt = sbn // 128
                    # matmul1: logitsT per v-tile, exp -> pt
                    pt = pt_p.tile([128, NVT, TSB_MAX], BF16, tag="pt",
                                   name=f"pt_{si}")
                    NQ = NJ // 2
                    for vt in range(NVT):
                        w2q = []
                        for q in range(2):
                            wq = w2_p.tile([128, NQ, 128], BF16, tag="w2",
                                           name=f"w2_{si}_{vt}_{q}")
                            nc.sync.dma_start(
                                wq, w2t[vt][:, NQ * q:NQ * (q + 1), :])
                            w2q.append(wq)
                        for c0, cw in C1S:
                            lp = l_ps.tile([128, 512], F32, tag="lp",
                                           name=f"lp_{si}_{vt}_{c0}")
                            for j in range(NJ):
                                nc.tensor.matmul(
                                    lp[:, :cw],
                                    lhsT=w2q[j // NQ][:, j % NQ, :],
                                    rhs=xnt[:, j, c0:c0 + cw],
                                    start=(j == 0), stop=(j == NJ - 1))
                            nc.scalar.activation(
                                out=pt[:, vt, c0:c0 + cw], in_=lp[:, :cw],
                                func=mybir.ActivationFunctionType.Exp)

                    # queue next superblock's xnt loads ahead of matmul2
                    next_xnt = make_xnt(si + 1) if si + 1 < len(SBS) else None

                    rs_rows = sbn // N_CORES

                    # matmul2: F_partial = pt.T @ emb, RS per e-group, local
                    # divide on owned rows. Last superblock's last group is
                    # host-reduced to shrink the exposed RS tail.
                    egroups = [2, 2, 2, 2]
                    last_host = si == len(SBS) - 1
                    row_off = sb0 // N_CORES
                    rsg = None
                    col = 0
                    e = 0
                    # denominator s[t] = sum over real v rows of pt (onesv
                    # masks the pads); done as N=1 matmul chains up front so
                    # it never interrupts the F-matmul pipeline, riding as an
                    # extra column of e-group 0's ReduceScatter.
                    gw0 = EG * EC + 1
                    rs_in0 = dram_rs.tile([TSB_MAX, gw0], BF16, tag="rsin",
                                          name=f"rsin_{si}_0")
                    for tt in range(n_tt):
                        sp = s_ps.tile([128, 1], F32, tag="sp",
                                       name=f"sp_{si}_{tt}")
                        for vt in range(NVT):
                            nc.tensor.matmul(
                                sp, lhsT=pt[:, vt, 128 * tt:128 * (tt + 1)],
                                rhs=onesv_sb[:, vt, :],
                                start=(vt == 0), stop=(vt == NVT - 1))
                        ss = fs_p.tile([128, 1], BF16, tag="fs",
                                       name=f"ss_{si}_{tt}")
                        nc.vector.tensor_copy(out=ss, in_=sp)
                        nc.sync.dma_start(
                            rs_in0[128 * tt:128 * (tt + 1), gw0 - 1:gw0], ss)
                        if last_host:
                            nc.sync.dma_start(
                                out3[128 * tt:128 * (tt + 1), :], ss)
                    for gi, gsz in enumerate(egroups):
                        # group 0 carries one extra column: the softmax
                        # denominator, so the division happens locally on
                        # owned rows after the ReduceScatter.
                        gw = gsz * EC + (1 if gi == 0 else 0)
                        if gi == 0:
                            rs_in = rs_in0
                        else:
                            rs_in = dram_rs.tile([TSB_MAX, gw], BF16,
                                                 tag="rsin",
                                                 name=f"rsin_{si}_{gi}")
                        for ei in range(gsz):
                            eb = eb_p.tile([128, NVT, EC], BF16, tag="eb",
                                           name=f"eb_{si}_{e}")
                            # split the 4MB load into 8 sub-DMAs so it
                            # spreads across queues instead of serializing
                            # on one
                            for sp8 in range(8):
                                nc.sync.dma_start(
                                    eb[:, 4 * sp8:4 * (sp8 + 1), :],
                                    emb[e][:, 4 * sp8:4 * (sp8 + 1), :])
                            for tt in range(n_tt):
                                fp = f_ps.tile([128, EC], F32, tag="fp",
                                               name=f"fp_{si}_{e}_{tt}")
                                for vt in range(NVT):
                                    nc.tensor.matmul(
                                        fp,
                                        lhsT=pt[:, vt, 128 * tt:128 * (tt + 1)],
                                        rhs=eb[:, vt, :],
                                        start=(vt == 0), stop=(vt == NVT - 1))
                                fs = fs_p.tile([128, EC], BF16, tag="fs2",
                                               name=f"fs_{si}_{e}_{tt}")
                                nc.vector.tensor_copy(out=fs, in_=fp)
                                if last_host and gi == len(egroups) - 1:
                                    nc.sync.dma_start(
                                        out2[128 * tt:128 * (tt + 1),
                                             EC * ei:EC * (ei + 1)], fs)
                                else:
                                    nc.sync.dma_start(
                                        rs_in[128 * tt:128 * (tt + 1),
                                              EC * ei:EC * (ei + 1)], fs)
                            e += 1
                        if last_host and gi == len(egroups) - 1:
                            col += gsz * EC
                            continue
                        rs_out = dram_rs.tile([TSB_MAX // N_CORES, gw],
                                              BF16, tag="rsout",
                                              name=f"rsout_{si}_{gi}")
                        nc.gpsimd.collective_compute(
                            "ReduceScatter", mybir.AluOpType.add,
                            replica_groups=rg,
                            ins=[rs_in.opt()],
                            outs=[rs_out.opt()])
                        fo = fo_p.tile([TSB_MAX // N_CORES, EGW + 1],
                                       BF16, tag="fo", name=f"fo_{si}_{gi}")
                        nc.sync.dma_start(fo[:rs_rows, :gw], rs_out[:])
                        fw = gsz * EC
                        if gi == 0:
                            rsg = small.tile([TSB_MAX // N_CORES, 1], F32,
                                             tag="rsg", name=f"rsg_{si}")
                            nc.vector.reciprocal(
                                out=rsg[:rs_rows],
                                in_=fo[:rs_rows, gw - 1:gw])
                        nc.vector.tensor_scalar_mul(
                            out=fo[:rs_rows, :fw], in0=fo[:rs_rows, :fw],
                            scalar1=rsg[:rs_rows])
                        nc.gpsimd.dma_start(
                            out[row_off:row_off + rs_rows, col:col + fw],
                            fo[:rs_rows, :fw])
                        col += fw
                    xnt = next_xnt

    nc.compile()
    return nc


def _get_nc():
    global _NC_CACHE
    if _NC_CACHE is None:
        _NC_CACHE = build()
    return _NC_CACHE


def _prep_in_maps(vision_feats, W1_w, W1_b, W2_w, llm_token_embed):
    vf = np.ascontiguousarray(np.asarray(vision_feats, np.float32)).reshape(
        T, DV)
    W1 = np.asarray(W1_w, np.float32)
    b1 = np.ascontiguousarray(np.asarray(W1_b, np.float32)).reshape(
        1, DL).astype(BF16NP)
    W2 = np.asarray(W2_w, np.float32)
    E = np.asarray(llm_token_embed, np.float32)

    w1t = np.ascontiguousarray(W1.T).astype(BF16NP)
    v_loc = 32000 // N_CORES
    in_maps = []
    for c in range(N_CORES):
        # strided ownership: core c owns tokens {768*si + 96*c + i}
        tok = np.concatenate([
            np.arange(sb0 + TCH * c, sb0 + TCH * (c + 1))
            for sb0, _ in SBS])
        vT = np.ascontiguousarray(vf[tok].T).astype(BF16NP)
        w2p = np.zeros((V_PAD, DL), np.float32)
        w2p[:v_loc] = W2[v_loc * c:v_loc * (c + 1)]
        # [vt, p, j, vi] with p = d % 128, j = d // 128, vi = v % 128
        w2tt = w2p.T.reshape(NJ, 128, NVT, 128).transpose(2, 1, 0, 3).astype(
            BF16NP)
        ep = np.zeros((V_PAD, DL), np.float32)
        ep[:v_loc] = E[v_loc * c:v_loc * (c + 1)]
        # [e, p, vt, n] with p = v % 128, vt = v // 128, n = d % EC
        ebt = ep.reshape(NVT, 128, N_EC, EC).transpose(2, 1, 0, 3).astype(
            BF16NP)
        onesv = np.zeros((128, NVT, 1), np.float32)
        for vt in range(NVT):
            for p in range(128):
                if 128 * vt + p < v_loc:
                    onesv[p, vt, 0] = 1.0
        in_maps.append({
            "visionT": vT,
            "w1t": w1t,
            "w1b": b1,
            "w2t": np.ascontiguousarray(w2tt),
            "emb": np.ascontiguousarray(ebt),
            "ones_v": onesv.astype(BF16NP),
        })
    return in_maps


def run_on_cores(in_maps, trace=False, **kwargs):
    nc = _get_nc()
    return run_bass_kernel_spmd(nc, in_maps, core_ids=list(range(N_CORES)),
                                trace=trace, **kwargs)


def assemble(core_results):
    full = np.empty((T, DL), np.float32)
    for c in range(N_CORES):
        o = np.asarray(core_results[c]["out"])  # [T // N_CORES, DL]
        for sb0, sbn in SBS:
            rs_rows = sbn // N_CORES
            full[sb0 + rs_rows * c:sb0 + rs_rows * (c + 1)] = \
                o[sb0 // N_CORES:sb0 // N_CORES + rs_rows]
    # host-reduced last e-group of the last superblock
    sb0, sbn = SBS[-1]
    num = sum(np.asarray(r["out2"]).astype(np.float32)
              for r in core_results)
    den = sum(np.asarray(r["out3"]).astype(np.float32)
              for r in core_results)
    full[sb0:sb0 + sbn, DL - EGW:] = num / den
    return full.reshape(4, 576, DL)


def kernel(**inputs):
    in_maps = _prep_in_maps(**inputs)
    res = run_on_cores(in_maps)
    return assemble(res.results)
